# revision 9
# baseline (speedup 1.0000x reference)
"""Trainium2 Bass kernel for CombinedBandPassFilterSequential.

Zero-phase (filtfilt-style) FIR filter bank: 10 phase bands (K=769) +
10 amplitude bands (K=129) over a single (1,1,2097152) fp32 signal;
output is the 20 band signals concatenated on the last axis.

Strategy
--------
Time-sharded SPMD over 8 NeuronCores: each core processes a contiguous
T/8 slice of the signal for ALL 20 bands (perfect load balance).

Every band is computed as a SINGLE fused correlation with the filter's
autocorrelation g = corr(h, h) (zero-phase transfer): pha bands get a
1537-tap g (13 Toeplitz chunks), amp bands a 257-tap g (3 chunks).
Each 1-D correlation is a sequence of 128x128 @ 128x512 tensor-engine
matmuls over banded-Toeplitz weight chunks in bf16 (fp32 PSUM accum):

  out[128*i + r] = sum_q  W_q[:, r] . x_cols[:, i + q - Q0]

where x_cols[p, m] = x[128*m + p] and W_q[p, r] = g[128*(q-Q0) + p - r + c].

The fused result differs from the reference's two-pass scheme only in
the first/last c = (K-1)/2 samples of the GLOBAL sequence (the 'SAME'
zero-pad crop of the intermediate).  Those 384 (pha) / 64 (amp) samples
per side get an exact precomputed correction, shipped as a per-core
input that is zero except on cores 0/7 (one SPMD program serves all
cores) and applied with two tiny vector subtracts per band.

A short burst of dummy warm-up matmuls runs while the first input DMAs
are in flight so the PE's HAM clock-gate is already at full rate when
real work arrives.
"""
import numpy as np
import ml_dtypes

import concourse.bass as bass
import concourse.tile as tile
from concourse import bacc, mybir
from concourse import bass_utils

# ---- problem geometry (hardcoded per contest rules) ----
T = 2097152
NCORES = 8
L = T // NCORES          # 262144 samples per core
LC = L // 128            # 2048 output columns per core
XH = 6                   # x halo columns each side (768 samples)
XC = LC + 2 * XH         # 2060 x columns
NB = 10                  # bands per filter group
QP, Q0P = 13, 6          # pha fused autocorr (1537 taps): chunks, offset
QA, Q0A = 3, 1           # amp fused autocorr (257 taps): chunks, offset
CP = 384                 # pha edge-correction width (3 cols)
CA = 64                  # amp edge-correction width
N = 512                  # matmul moving width
NG = LC // N             # 4 output groups per band
NWARM = 5                # warm-up matmuls

F32 = mybir.dt.float32
BF16 = mybir.dt.bfloat16
BFNP = ml_dtypes.bfloat16


def _toeplitz_chunks(h, Q0, NQ):
    """W[q][p, r] = h[128*(q - Q0) + p - r + c], zero outside [0, K)."""
    h = np.asarray(h, np.float64)
    K = len(h)
    c = (K - 1) // 2
    W = np.zeros((NQ, 128, 128), np.float64)
    p = np.arange(128)[:, None]
    r = np.arange(128)[None, :]
    for q in range(NQ):
        k = 128 * (q - Q0) + p - r + c
        valid = (k >= 0) & (k < K)
        W[q][valid] = h[np.clip(k, 0, K - 1)][valid]
    return W


def _head_D(h, xh):
    """fused - exact on the first c samples; xh = x[:3c] (float64).

    exact: two-pass zero-phase with the intermediate cropped to [0, T)
    (the reference's 'SAME' scheme); fused: correlation with autocorr(h).
    """
    h = np.asarray(h, np.float64)
    K = len(h)
    c = (K - 1) // 2
    xp = np.concatenate([np.zeros(c), xh])           # xp[m] = x[m - c]
    y1 = np.correlate(xp, h, 'valid')                # y1[t], t in [0, 2c)
    z = np.concatenate([np.zeros(c), y1])            # z[m] = y1[m - c]
    yex = np.convolve(h, z)[2 * c: 3 * c]            # exact y[0:c]
    g = np.correlate(h, h, 'full')                   # 2K-1 taps
    xq = np.concatenate([np.zeros(2 * c), xh])       # xq[m] = x[m - 2c]
    f = np.correlate(xq, g, 'valid')[:c]             # fused y[0:c]
    return f - yex


def _build_program():
    nc = bacc.Bacc("TRN2", target_bir_lowering=False, debug=False,
                   enable_asserts=True, num_devices=NCORES)

    x_ap = nc.dram_tensor("xT", [128, XC], BF16, kind="ExternalInput").ap()
    wp_ap = nc.dram_tensor("wp", [128, NB * QP * 128], BF16,
                           kind="ExternalInput").ap()
    wa_ap = nc.dram_tensor("wa", [128, NB * QA * 128], BF16,
                           kind="ExternalInput").ap()
    ch_ap = nc.dram_tensor("corr_h", [128, 2 * NB * 3], F32,
                           kind="ExternalInput").ap()
    ct_ap = nc.dram_tensor("corr_t", [128, 2 * NB * 3], F32,
                           kind="ExternalInput").ap()
    out_ap = nc.dram_tensor("out", [2 * NB, 128, LC], F32,
                            kind="ExternalOutput").ap()

    with tile.TileContext(nc) as tc:
        with tc.tile_pool(name="const", bufs=1) as cpool, \
             tc.tile_pool(name="psum", bufs=6, space="PSUM") as psum_pool, \
             tc.tile_pool(name="psumw", bufs=1, space="PSUM") as psumw_pool, \
             tc.tile_pool(name="stage", bufs=4) as stage_pool:

            xt = cpool.tile([128, XC], BF16, name="xt", tag="x")
            wp = cpool.tile([128, NB * QP * 128], BF16, name="wpt", tag="wp")
            wa = cpool.tile([128, NB * QA * 128], BF16, name="wat", tag="wa")
            ch = cpool.tile([128, 2 * NB * 3], F32, name="cht", tag="ch")
            ct = cpool.tile([128, 2 * NB * 3], F32, name="ctt", tag="ct")
            wrm = cpool.tile([128, N], BF16, name="wrm", tag="warm")

            # ---- PE warm-up while the first input DMAs are in flight ----
            nc.gpsimd.memset(wrm[:], 0.0)
            pw = psumw_pool.tile([128, N], F32, tag="pw")
            for _ in range(NWARM):
                nc.tensor.matmul(pw[:], wrm[:, :128], wrm[:],
                                 start=True, stop=True)

            # ---- input DMAs, ordered by first use ----
            # sync (SP) ring: x chunks and per-band amp weights interleaved,
            # then corrections.  scalar (ACT) ring: pha weights band by band.
            def wa_slice(b):
                s, e = b * QA * 128, (b + 1) * QA * 128
                nc.sync.dma_start(wa[:, s:e], wa_ap[:, s:e])

            nc.sync.dma_start(xt[:, 0:272], x_ap[:, 0:272])
            wa_slice(0)
            nc.sync.dma_start(xt[:, 272:528], x_ap[:, 272:528])
            nc.sync.dma_start(xt[:, 528:1040], x_ap[:, 528:1040])
            wa_slice(1)
            wa_slice(2)
            nc.sync.dma_start(xt[:, 1040:1552], x_ap[:, 1040:1552])
            for b in range(3, 6):
                wa_slice(b)
            nc.sync.dma_start(xt[:, 1552:XC], x_ap[:, 1552:XC])
            for b in range(6, NB):
                wa_slice(b)
            nc.sync.dma_start(ch[:], ch_ap[:])
            nc.sync.dma_start(ct[:], ct_ap[:])
            for b in range(NB):
                s = b * QP * 128
                e = (b + 1) * QP * 128
                nc.scalar.dma_start(wp[:, s:e], wp_ap[:, s:e])

            ncopy = [0]

            def drain(ps, band_out, g, cb, cbt):
                """PSUM -> SBUF (alternating DVE/ACT), edge-fix, DMA out."""
                st = stage_pool.tile([128, N], F32, tag="st")
                if ncopy[0] % 2 == 0:
                    nc.vector.tensor_copy(st[:], ps[:])
                else:
                    nc.scalar.copy(st[:], ps[:])
                ncopy[0] += 1
                if g == 0:
                    nc.vector.tensor_sub(st[:, 0:3], st[:, 0:3],
                                         ch[:, cb:cb + 3])
                if g == NG - 1:
                    nc.vector.tensor_sub(st[:, N - 3:], st[:, N - 3:],
                                         ct[:, cbt:cbt + 3])
                nc.sync.dma_start(out_ap[band_out, :, g * N:(g + 1) * N],
                                  st[:])

            def amp_group(b, g, split=False):
                ps = psum_pool.tile([128, N], F32, tag="ps")
                # split=True: two half-width accumulation groups so the very
                # first matmuls only need the first small x chunk
                for s0, sn in ([(0, N // 2), (N // 2, N // 2)] if split
                               else [(0, N)]):
                    for q in range(QA):
                        m0 = XH + g * N + s0 + q - Q0A
                        nc.tensor.matmul(
                            ps[:, s0:s0 + sn],
                            wa[:, (b * QA + q) * 128:(b * QA + q + 1) * 128],
                            xt[:, m0:m0 + sn],
                            start=(q == 0), stop=(q == QA - 1),
                        )
                drain(ps, NB + b, g, (NB + b) * 3, (NB + b) * 3)

            def pha_group(b, g):
                ps = psum_pool.tile([128, N], F32, tag="ps")
                for q in range(QP):
                    m0 = XH + g * N + q - Q0P
                    nc.tensor.matmul(
                        ps[:],
                        wp[:, (b * QP + q) * 128:(b * QP + q + 1) * 128],
                        xt[:, m0:m0 + N],
                        start=(q == 0), stop=(q == QP - 1),
                    )
                drain(ps, b, g, b * 3, b * 3)

            # ---- strict 1:1 interleave: each short (3-matmul) amp group
            # drains inside the next long (13-matmul) pha accumulation, so
            # PSUM drains never gate the PE ----
            amp_group(0, 0, split=True)
            for idx in range(NB * NG):
                pha_group(idx // NG, idx % NG)
                if idx + 1 < NB * NG:
                    amp_group((idx + 1) // NG, (idx + 1) % NG)

    nc.compile()
    return nc


_CACHE = {}


def _get_program():
    if "nc" not in _CACHE:
        _CACHE["nc"] = _build_program()
    return _CACHE["nc"]


def _host_inputs(x, pha_filters, amp_filters):
    x = np.ascontiguousarray(np.asarray(x, np.float32).reshape(T))
    pha = np.asarray(pha_filters, np.float64)
    amp = np.asarray(amp_filters, np.float64)

    gp = [np.correlate(h, h, 'full') for h in pha]   # 1537 taps
    ga = [np.correlate(h, h, 'full') for h in amp]   # 257 taps
    wp = np.stack([_toeplitz_chunks(g, Q0P, QP) for g in gp])
    wa = np.stack([_toeplitz_chunks(g, Q0A, QA) for g in ga])

    def wlay(W):  # (NB, NQ, 128p, 128r) -> (128p, NB*NQ*128r) bf16
        return np.ascontiguousarray(
            W.transpose(2, 0, 1, 3).reshape(128, -1).astype(BFNP))

    wp, wa = wlay(wp), wlay(wa)

    x64 = x.astype(np.float64)
    # edge corrections: D = fused - exact (fp64), head and tail per band
    ch = np.zeros((128, 2 * NB * 3), np.float32)
    ct = np.zeros((128, 2 * NB * 3), np.float32)
    for b in range(NB):
        dh = _head_D(pha[b], x64[:3 * CP])
        dt = _head_D(pha[b][::-1], x64[T - 3 * CP:][::-1])[::-1]
        ch[:, 3 * b:3 * b + 3] = dh.reshape(3, 128).T
        ct[:, 3 * b:3 * b + 3] = dt.reshape(3, 128).T
    for b in range(NB):
        dh = _head_D(amp[b], x64[:3 * CA])          # (64,)
        dt = _head_D(amp[b][::-1], x64[T - 3 * CA:][::-1])[::-1]
        ch[:CA, 3 * (NB + b)] = dh
        ct[CA:2 * CA, 3 * (NB + b) + 2] = dt
    zeros = np.zeros_like(ch)

    xp = np.zeros(T + 2 * XH * 128, np.float32)
    xp[XH * 128: XH * 128 + T] = x

    in_maps = []
    for c in range(NCORES):
        n0 = c * L
        xT = np.ascontiguousarray(
            xp[n0:n0 + L + 2 * XH * 128].reshape(XC, 128).T.astype(BFNP))
        in_maps.append({
            "xT": xT,
            "wp": wp, "wa": wa,
            "corr_h": ch if c == 0 else zeros,
            "corr_t": ct if c == NCORES - 1 else zeros,
        })
    return in_maps


def _gather(results):
    out = np.empty((2 * NB, T), np.float32)
    for c in range(NCORES):
        oc = results[c]["out"]  # (20, 128, LC): [band, r, i] = y[128*i + r]
        out[:, c * L:(c + 1) * L] = oc.transpose(0, 2, 1).reshape(2 * NB, L)
    return out.reshape(1, 1, 2 * NB * T)


def run(x, pha_filters, amp_filters, trace=False):
    nc = _get_program()
    in_maps = _host_inputs(x, pha_filters, amp_filters)
    res = bass_utils.run_bass_kernel_spmd(
        nc, in_maps, core_ids=list(range(NCORES)), trace=trace)
    return _gather(res.results), res


def kernel(x, pha_filters, amp_filters):
    out, _ = run(x, pha_filters, amp_filters)
    return out
